# revision 51
# baseline (speedup 1.0000x reference)
"""Trainium2 Bass kernel for nn_CGLayer (gnn_message_passing).

Contract: kernel(**inputs) takes FULL inputs (as reference.setup_inputs()),
returns FULL output [8,128,1,16,9] f32. Internally: data-parallel over the
batch dim across 8 NeuronCores; per core one batch element.

Algebraic reduction (exact):
  X   = conn @ vertices                  (message passing, per batch)
  Y   = mix_nl(cg(X, X))                 (per-node quadratic in X)
  S   = sum_j sph[:, j, :]               (neighbor sum commutes through the
  Z   = mix_rel(cg(Y, S))                 relative-CG stage: x-side is
  out = Z / sqrt(sum Z^2 / 16)            j-independent)

Device pipeline per core (bf16 engines, f32 PSUM accumulate):
  A:  X[i,144]     = matmul(lhsT=connT, rhs=vcat)
  B:  P[i,9984]    = 13 tensor_tensor product ops split DVE/GpSimd (dense
                     q-major slots, l1==l2 m-pairs folded by symmetry)
      PT[s,tp,i]   = 5 grouped XBAR dma_start_transposes of P (single HWDGE
                     queue: concurrent XBAR transposes corrupt; drain fences
                     give consumers completion ordering)
      YT[i, r]     = 78 matmuls lhsT=PT-chunk rhs=W2-chunk, PSUM-accumulated;
                     one PSUM bank per s-group so mixes run in transpose
                     completion order (interleaved open accumulation groups
                     sharing a bank break)
      Y rows       = XBAR transpose of YT back to (g,a)-partition layout
  S:  Ssum=reduce_j(sph) -> XBAR -> DRAM -> broadcast SREP128[.,n,i]
  C:  P2 = YTT * SREP (DVE 2x); Z[i,144] += P2_n.T @ W3_n  (18 matmuls)
Host epilogue: gather, unpack e=(l,c',k), global normalization per l.
"""
import numpy as np
from math import factorial, sqrt

MAXL = 2
CH = 16
NN = 128
NB = 8
LDIM = [1, 3, 5]
FOFF = [0, 16, 64]
NF = 144

# ------------------------------------------------------------- CG tables
def _cg_coeff(j1, m1, j2, m2, j3, m3):
    if m3 != m1 + m2:
        return 0.0
    pre = sqrt((2 * j3 + 1) * factorial(j3 + j1 - j2) * factorial(j3 - j1 + j2)
               * factorial(j1 + j2 - j3) / factorial(j1 + j2 + j3 + 1))
    pre *= sqrt(factorial(j3 + m3) * factorial(j3 - m3) * factorial(j1 - m1)
                * factorial(j1 + m1) * factorial(j2 - m2) * factorial(j2 + m2))
    s = 0.0
    vmin = max(0, j2 - j3 - m1, j1 - j3 + m2)
    vmax = min(j1 + j2 - j3, j1 - m1, j2 + m2)
    for v in range(vmin, vmax + 1):
        s += (-1) ** v / (factorial(v) * factorial(j1 + j2 - j3 - v)
                          * factorial(j1 - m1 - v) * factorial(j2 + m2 - v)
                          * factorial(j3 - j2 + m1 + v) * factorial(j3 - j1 - m2 + v))
    return pre * s


def _cg_matrix(l1, l2, l):
    M = np.zeros((2 * l1 + 1, 2 * l2 + 1, 2 * l + 1))
    for m1 in range(-l1, l1 + 1):
        for m2 in range(-l2, l2 + 1):
            if -l <= m1 + m2 <= l:
                M[m1 + l1, m2 + l2, m1 + m2 + l] = _cg_coeff(l1, m1, l2, m2, l, m1 + m2)
    return M


def _valid_pairs(l):
    return [(l1, l2) for l1 in range(3) for l2 in range(3)
            if abs(l1 - l2) <= l <= l1 + l2]

# ----------------------------------------------------- stage-B slot layout
def _build_q():
    qs = []
    for l1 in range(3):
        for l2 in range(l1, 3):
            for m1 in range(2 * l1 + 1):
                mt1 = m1 - l1
                m2_lo = max(0, -2 - mt1 + l2)
                m2_hi = min(2 * l2, 2 - mt1 + l2)
                if l1 == l2:
                    m2_lo = max(m2_lo, m1)
                if m2_lo > m2_hi:
                    continue
                qs.append((l1, l2, m1, m2_lo, m2_hi))
    return qs

QS = _build_q()                                          # 13 product ops
QOFF = np.concatenate([[0], np.cumsum([(q[4] - q[3] + 1) * 256 for q in QS])])
NSLOT = int(QOFF[-1])                                    # 9984
NCHUNK = NSLOT // 128                                    # 78

_DOFF = {}
_BLOCK_OF_CHUNK = []
CH_Q = []
for _k, (_l1, _l2, _m1, _lo, _hi) in enumerate(QS):
    for _m2 in range(_lo, _hi + 1):
        _DOFF[(_l1, _l2, _m1, _m2)] = int(QOFF[_k]) + (_m2 - _lo) * 256
        _BLOCK_OF_CHUNK += [(_l1, _l2, _m1, _m2)] * 2
        CH_Q += [_k, _k]
G_OF_CHUNK = [(m1 - l1) + (m2 - l2) + 2 for (l1, l2, m1, m2) in _BLOCK_OF_CHUNK]

# engine split and emission order (measured: DVE ~1.6 ns/col, Pool ~3.9)
DVE_QS = [2, 7, 6, 10, 3, 9, 1]
POOL_QS = [8, 11, 4, 0, 5, 12]
# XBAR transpose groups (engine, [q...]); concat per engine == emission order
TGROUPS = [(0, [2, 7]), (0, [6, 10]), (0, [3, 9, 1]), (1, [8, 11]), (1, [4, 0, 5, 12])]

def _nslots(k):
    return (QS[k][4] - QS[k][3] + 1) * 256

_PLOC = {}
_PSZ = [0, 0]
for _e, _qs in ((0, DVE_QS), (1, POOL_QS)):
    for _k in _qs:
        _PLOC[_k] = (_e, _PSZ[_e])
        _PSZ[_e] += _nslots(_k)

TBASE = {}
_tp = 0
for _e, _qs in TGROUPS:
    for _k in _qs:
        TBASE[_k] = _tp
        _tp += _nslots(_k) // 128
TPOS = [TBASE[CH_Q[c]] + (c - int(QOFF[CH_Q[c]]) // 128) for c in range(NCHUNK)]
MIX_ORDER = [c for _e, _qs in TGROUPS for _k in _qs
             for c in range(int(QOFF[_k]) // 128, int(QOFF[_k + 1]) // 128)]
_GPOS = {}
for _i, _c in enumerate(MIX_ORDER):
    _g = G_OF_CHUNK[_c]
    _GPOS.setdefault(_g, []).append(_i)
G_START = {g: v[0] for g, v in _GPOS.items()}
G_STOP = {g: v[-1] for g, v in _GPOS.items()}


def _sgroup_cols(g):
    st = g - 2
    return [(l, cp) for l in range(3) if abs(st) <= l for cp in range(CH)]

SG_NCOL = [len(_sgroup_cols(g)) for g in range(5)]      # [16,32,48,32,16]
YOFF = np.concatenate([[0], np.cumsum(SG_NCOL)])        # Y piece row offsets


def _sg_lblock_col(g, l):
    st = g - 2
    return 16 * sum(1 for lp in range(l) if abs(st) <= lp)

_CAR, _DAR = np.meshgrid(np.arange(16), np.arange(16), indexing="ij")


def _assemble_W2(w_nl):
    """W2[NSLOT, 48] f64: combined CG x w_nl, dense folded layout."""
    W2 = np.zeros((NSLOT, 48))
    for l in range(3):
        off = 0
        for (p1, p2) in _valid_pairs(l):
            Cg = _cg_matrix(p1, p2, l)
            wl = np.asarray(w_nl[l], np.float64)
            for m1 in range(2 * p1 + 1):
                for m2 in range(2 * p2 + 1):
                    st = (m1 - p1) + (m2 - p2)
                    if abs(st) > l:
                        continue
                    gc = Cg[m1, m2, st + l]
                    if gc == 0.0:
                        continue
                    g = st + 2
                    if p1 < p2 or (p1 == p2 and m1 <= m2):
                        slots = _DOFF[(p1, p2, m1, m2)] + _CAR * 16 + _DAR
                    else:
                        slots = _DOFF[(p2, p1, m2, m1)] + _DAR * 16 + _CAR
                    t = off + _CAR * 16 + _DAR
                    c0 = _sg_lblock_col(g, l)
                    W2[slots.ravel(), c0:c0 + 16] += gc * wl[t.ravel(), :]
            off += 256
    return W2


def _assemble_W3(w_rel):
    """W3[9*144, 144]: contraction P2[(n,r), i] -> Z[e, i]; r = Y piece row."""
    SOFF = [0, 1, 4]
    W3 = np.zeros((9 * 144, 144))
    ar = np.arange(16)
    for l in range(3):
        off = 0
        for (p1, p2) in _valid_pairs(l):
            Cg = _cg_matrix(p1, p2, l)
            wr = np.asarray(w_rel[l], np.float64)
            for m1 in range(2 * p1 + 1):
                for m2 in range(2 * p2 + 1):
                    st = (m1 - p1) + (m2 - p2)
                    if abs(st) > l:
                        continue
                    gc = Cg[m1, m2, st + l]
                    if gc == 0.0:
                        continue
                    gY = (m1 - p1) + 2
                    a0 = YOFF[gY] + _sg_lblock_col(gY, p1)
                    rows = (SOFF[p2] + m2) * 144 + a0 + ar
                    cols = FOFF[l] + (st + l) + ar * LDIM[l]
                    W3[np.ix_(rows, cols)] += gc * wr[off:off + 16, :]
            off += 16
    return W3

# ------------------------------------------------------------ bass builder
_NC_CACHE = {}


def _build_nc(debug=False):
    import concourse.bacc as bacc
    import concourse.bass as bass
    import concourse.tile as tile
    from concourse import mybir
    from concourse.tile import add_dep_helper

    def dep(a, b, why):
        # annotate_deps misses InstDmaTransposeAnt / hand-built AP operands;
        # wire edges explicitly (drain fences give DMA-completion ordering).
        add_dep_helper(a.ins, b.ins, reason=why)

    f32 = mybir.dt.float32
    bf16 = mybir.dt.bfloat16
    nc = bacc.Bacc()
    d_cv = nc.declare_dram_parameter("cvcat", [128, 128 + NF], bf16, isOutput=False)
    d_sph = nc.declare_dram_parameter("sph", [128, 128 * 9], bf16, isOutput=False)
    d_w2 = nc.declare_dram_parameter("w2", [128, NCHUNK * 48], bf16, isOutput=False)
    d_w3a = nc.declare_dram_parameter("w3a", [128, 9 * 144], bf16, isOutput=False)
    d_w3b = nc.declare_dram_parameter("w3b", [48, 9 * 144], bf16, isOutput=False)
    d_zout = nc.declare_dram_parameter("zout", [128, NF], f32, isOutput=True)
    if debug:
        d_dbgx = nc.declare_dram_parameter("dbgx", [128, NF], f32, isOutput=True)
        d_dbgs = nc.declare_dram_parameter("dbgs", [128, 9 * 128], f32, isOutput=True)
        d_dbgp = nc.declare_dram_parameter("dbgp", [128, NSLOT], f32, isOutput=True)
        d_dbgy = nc.declare_dram_parameter("dbgy", [128, 256], f32, isOutput=True)

    def vap(t, doff, freedims):
        base = t[:] if not isinstance(t, bass.AP) else t
        return bass.AP(tensor=base.tensor, offset=base.offset + doff,
                       ap=[list(base.ap[0])] + [list(d) for d in freedims])

    with tile.TileContext(nc) as tc:
      with (
        tc.tile_pool(name="big", bufs=1) as big,
        tc.tile_pool(name="sb", bufs=1) as sb,
        tc.tile_pool(name="ps_m", bufs=1, space="PSUM") as ps_m,
        tc.tile_pool(name="ps_g", bufs=1, space="PSUM") as ps_g,
      ):
        # ---- input DMAs on ACT queue (keeps the SP HWDGE free for XBAR)
        cv = sb.tile([128, 128 + NF], bf16)
        nc.scalar.dma_start(out=cv, in_=d_cv[:, :])

        # ---- stage A: X[i, feat] = connT.T @ vcat
        x_ps = ps_m.tile([128, NF], f32, tag="misc", name="x_ps")
        nc.tensor.matmul(x_ps, cv[:, 0:128], cv[:, 128:128 + NF],
                         start=True, stop=True)
        X = sb.tile([128, NF], bf16)
        nc.scalar.activation(X, x_ps, mybir.ActivationFunctionType.Copy)

        sph = big.tile([128, 128 * 9], bf16)   # [j, (n, i)] layout
        nc.scalar.dma_start(out=sph, in_=d_sph[:, :])
        w2 = big.tile([128, NCHUNK, 48], bf16)
        for q in range(2):
            lo = (NCHUNK * q // 2) * 48
            hi = (NCHUNK * (q + 1) // 2) * 48
            nc.scalar.dma_start(out=vap(w2, lo, [[1, hi - lo]]), in_=d_w2[:, lo:hi])
        w3a = sb.tile([128, 9, 144], bf16)
        nc.scalar.dma_start(
            out=w3a, in_=d_w3a[:, :].rearrange("p (n e) -> p n e", n=9))
        w3b = sb.tile([48, 9, 144], bf16)
        nc.scalar.dma_start(
            out=w3b, in_=d_w3b[:, :].rearrange("p (n e) -> p n e", n=9))

        # ---- stage B products: P split into per-engine tiles (DVE / GpSimd)
        Pt = [big.tile([128, _PSZ[0]], bf16, name="P_dve"),
              big.tile([128, _PSZ[1]], bf16, name="P_pool")]
        pinst = {}
        for e, qs in ((0, DVE_QS), (1, POOL_QS)):
            eng = nc.vector if e == 0 else nc.gpsimd
            for k in qs:
                l1, l2, m1, m2_lo, m2_hi = QS[k]
                nm2 = m2_hi - m2_lo + 1
                loc = _PLOC[k][1]
                pinst[k] = eng.tensor_tensor(
                    out=vap(Pt[e], loc, [[256, nm2], [16, 16], [1, 16]]),
                    in0=vap(X, FOFF[l1] + m1, [[0, nm2], [LDIM[l1], 16], [0, 16]]),
                    in1=vap(X, FOFF[l2] + m2_lo,
                            [[1, nm2], [0, 16], [LDIM[l2], 16]]),
                    op=mybir.AluOpType.mult)

        # ---- XBAR transposes (single SP queue). Same-queue DMA transfers
        # execute in order, so a tiny regular DMA ("sliver fence") right after
        # each transpose rewrites one element per chunk in place: tile tracks
        # it normally, giving the mixes completion-ordered RAW deps that
        # transitively cover the untracked XBAR write.
        PT = big.tile([128, NCHUNK, 128], bf16)
        gfence = []
        for e, qs in TGROUPS:
            lo = _PLOC[qs[0]][1]
            sz = sum(_nslots(k) for k in qs)
            t0 = TBASE[qs[0]]
            nch = sz // 128
            ti = nc.sync.dma_start_transpose(
                out=PT[:, t0:t0 + nch, :], in_=Pt[e][:, lo:lo + sz])
            for k in qs:
                dep(ti, pinst[k], "xbar reads P block")
            sliver = vap(PT, t0 * 128, [[128, nch]])
            fi = nc.sync.dma_start(out=sliver, in_=sliver)
            dep(fi, ti, "fence follows xbar on queue")
            gfence.append(fi)

        # ---- stage S on PE (idle window): S[n,i] = sum_j sph[j,(n,i)] via a
        # ones-matmul, then K=1 ones-matmul broadcasts it to 128 partitions.
        ones16 = sb.tile([128, 16], bf16)
        nc.gpsimd.memset(ones16, 1.0)
        ones1 = sb.tile([1, 128], bf16)
        nc.gpsimd.memset(ones1, 1.0)
        s_row = sb.tile([1, 9 * 128], bf16)
        SREP = sb.tile([128, 9, 128], bf16)
        for t in range(3):
            lo = t * 384
            s_ps = ps_m.tile([16, 384], f32, tag="sjs", name="s_ps")
            nc.tensor.matmul(s_ps, ones16, sph[:, lo:lo + 384],
                             start=True, stop=True)
            nc.scalar.activation(s_row[0:1, lo:lo + 384], s_ps[0:1, :],
                                 mybir.ActivationFunctionType.Copy)
            s_bc = ps_m.tile([128, 384], f32, tag="sbc", name="s_bc")
            nc.tensor.matmul(s_bc, ones1, s_row[0:1, lo:lo + 384],
                             start=True, stop=True)
            nc.scalar.activation(vap(SREP, lo, [[1, 384]]), s_bc,
                                 mybir.ActivationFunctionType.Copy)

        # ---- stage B mixes: Y rows stacked on PSUM partitions (bank
        # accumulation state is per-partition, so partition-disjoint groups
        # interleave freely). Matmul out base partition must be 0/32/64:
        # g0@A0, g1@A32, g2@A64, g3@B0, g4@B32; junk rows zeroed.
        ymA = ps_g.tile([128, 128], f32, name="ymA")
        ymB = ps_g.tile([48, 128], f32, name="ymB")
        # zero junk rows 16:32 and 112:128 (32-aligned ranges; runs before the
        # mixes' start=True overwrites, so touching real rows is harmless)
        nc.vector.memset(ymA[0:32, :], 0.0)
        nc.vector.memset(ymA[96:128, :], 0.0)
        G_ROW = {0: (0, 0), 1: (0, 32), 2: (0, 64), 3: (1, 0), 4: (1, 32)}
        for i, ch in enumerate(MIX_ORDER):
            g = G_OF_CHUNK[ch]
            ncol = SG_NCOL[g]
            t, base = G_ROW[g]
            out = (ymA if t == 0 else ymB)[base:base + ncol, :]
            mm = nc.tensor.matmul(
                out, w2[:, i, 0:ncol], PT[:, TPOS[ch], :],
                start=(i == G_START[g]), stop=(i == G_STOP[g]))
            if i in _MIXFENCE:
                dep(mm, gfence[_MIXFENCE[i]], "mix waits PT xbar drain")

        ysbA = sb.tile([128, 128], bf16)
        nc.scalar.activation(ysbA, ymA, mybir.ActivationFunctionType.Copy)
        ysbB = sb.tile([48, 128], bf16)
        nc.scalar.activation(ysbB, ymB, mybir.ActivationFunctionType.Copy)

        if debug:
            xdb = sb.tile([128, NF], f32)
            nc.vector.tensor_copy(out=xdb, in_=X)
            nc.sync.dma_start(out=d_dbgx[:, :], in_=xdb)
            sdb = big.tile([128, 9 * 128], f32)
            nc.vector.tensor_copy(out=sdb, in_=vap(SREP, 0, [[1, 9 * 128]]))
            nc.sync.dma_start(out=d_dbgs[:, :], in_=sdb)
            pdb = big.tile([128, NSLOT], f32)
            for k in range(len(QS)):
                e, loc = _PLOC[k]
                lo, hi = int(QOFF[k]), int(QOFF[k + 1])
                nc.vector.tensor_copy(out=vap(pdb, lo, [[1, hi - lo]]),
                                      in_=Pt[e][:, loc:loc + hi - lo])
            for q in range(4):
                lo = NSLOT * q // 4
                hi = NSLOT * (q + 1) // 4
                nc.sync.dma_start(out=d_dbgp[:, lo:hi],
                                  in_=vap(pdb, lo, [[1, hi - lo]]))
            ydb = sb.tile([128, 256], f32)
            nc.vector.tensor_copy(out=ydb[:, 0:128], in_=ysbA)
            nc.vector.tensor_copy(out=ydb[0:16, 128:256], in_=ysbB)
            nc.sync.dma_start(out=d_dbgy[:, :], in_=ydb)

        # ---- stage C: P2 = Y * SREP (DVE 2x); Z[i,144] += P2_n.T @ W3_n
        p2a = sb.tile([128, 9, 128], bf16)
        p2b = sb.tile([48, 9, 128], bf16)
        for n in range(9):
            nc.vector.tensor_tensor(
                out=p2a[:, n, :], in0=ysbA, in1=SREP[:, n, :],
                op=mybir.AluOpType.mult)
            nc.vector.tensor_tensor(
                out=p2b[:, n, :], in0=ysbB, in1=SREP[0:48, n, :],
                op=mybir.AluOpType.mult)
        zps = ps_m.tile([128, NF], f32, tag="misc", name="z_ps")
        for n in range(9):
            nc.tensor.matmul(zps, p2a[:, n, :], w3a[:, n, :],
                             start=(n == 0), stop=False)
            nc.tensor.matmul(zps, p2b[0:48, n, :], w3b[0:48, n, :],
                             start=False, stop=(n == 8))

        zs = sb.tile([128, NF], f32)
        nc.scalar.activation(zs, zps, mybir.ActivationFunctionType.Copy)
        nc.sync.dma_start(out=d_zout[:, :], in_=zs)

    nc.compile()
    return nc


# first mix position of each transpose group -> group fence index
_MIXFENCE = {}
_pos = 0
for _gi, (_e, _qs) in enumerate(TGROUPS):
    _MIXFENCE[_pos] = _gi
    _pos += sum(_nslots(_k) // 128 for _k in _qs)

# ------------------------------------------------------------- host entry
def _get_nc(debug=False):
    key = ("dbg" if debug else "nc")
    if key not in _NC_CACHE:
        _NC_CACHE[key] = _build_nc(debug)
    return _NC_CACHE[key]


def kernel(vertices_0, vertices_1, vertices_2, connectivity,
           sph_0, sph_1, sph_2,
           w_nl_0, w_nl_1, w_nl_2,
           w_rel_0, w_rel_1, w_rel_2, _debug=False):
    from concourse.bass_utils import run_bass_kernel_spmd
    import ml_dtypes

    f = np.float32
    bf = ml_dtypes.bfloat16
    verts = [np.asarray(v, f) for v in (vertices_0, vertices_1, vertices_2)]
    sphs = [np.asarray(s, f) for s in (sph_0, sph_1, sph_2)]
    conn = np.asarray(connectivity)
    W2 = _assemble_W2([np.asarray(w, f) for w in (w_nl_0, w_nl_1, w_nl_2)])
    W3 = _assemble_W3([np.asarray(w, f) for w in (w_rel_0, w_rel_1, w_rel_2)])
    # pack to SBUF-ready layouts (shared across cores); w2 chunks in MIX_ORDER
    w2p = np.ascontiguousarray(
        W2.reshape(NCHUNK, 128, 48)[MIX_ORDER].transpose(1, 0, 2)
        .reshape(128, NCHUNK * 48)).astype(bf)
    W3r = W3.reshape(9, 144, 144)
    # padded Y-row map: A rows [0:16]=g0, [32:64]=g1, [64:112]=g2 (junk rows
    # zeroed on device); B rows [0:32]=g3, [32:48]=g4
    w3a_r = np.zeros((128, 9, 144), np.float64)
    w3a_r[0:16] = W3r[:, 0:16, :].transpose(1, 0, 2)
    w3a_r[32:64] = W3r[:, 16:48, :].transpose(1, 0, 2)
    w3a_r[64:112] = W3r[:, 48:96, :].transpose(1, 0, 2)
    w3a = np.ascontiguousarray(w3a_r.reshape(128, 9 * 144)).astype(bf)
    w3b_r = W3r[:, 96:144, :].transpose(1, 0, 2)
    w3b = np.ascontiguousarray(w3b_r.reshape(48, 9 * 144)).astype(bf)

    in_maps = []
    for b in range(NB):
        connT = np.ascontiguousarray(conn[b].astype(f).T)
        vcat = np.concatenate([v[b].reshape(128, -1) for v in verts], axis=1)
        cvcat = np.concatenate([connT, vcat], axis=1).astype(bf)
        sph_cat = np.concatenate([s[b][:, :, 0, :] for s in sphs], axis=-1)
        sphT = sph_cat.transpose(1, 2, 0).reshape(128, 9 * 128)   # [j, (n, i)]
        in_maps.append(dict(cvcat=np.ascontiguousarray(cvcat),
                            sph=np.ascontiguousarray(sphT).astype(bf),
                            w2=w2p, w3a=w3a, w3b=w3b))

    res = run_bass_kernel_spmd(_get_nc(_debug), in_maps, list(range(NB)))
    if _debug:
        kernel._dbg = res
    Z = np.stack([res.results[b]["zout"] for b in range(NB)])   # [8, 128, 144]

    # host epilogue: unpack e=(l,cp,k) cols, global per-l normalization
    out = np.zeros((NB, 128, 1, 16, 9), dtype=f)
    koff = [0, 1, 4]
    for l in range(3):
        blk = Z[:, :, FOFF[l]:FOFF[l] + 16 * LDIM[l]]
        blk = blk.reshape(NB, 128, 16, LDIM[l])
        nf = np.sum(blk.astype(np.float64) ** 2)
        out[:, :, 0, :, koff[l]:koff[l] + LDIM[l]] = blk / np.sqrt(nf / 16.0)
    return out


# revision 52
# speedup vs baseline: 1.1013x; 1.1013x over previous
"""Trainium2 Bass kernel for nn_CGLayer (gnn_message_passing).

Contract: kernel(**inputs) takes FULL inputs (as reference.setup_inputs()),
returns FULL output [8,128,1,16,9] f32. Internally: data-parallel over the
batch dim across 8 NeuronCores; per core one batch element.

Algebraic reduction (exact):
  X   = conn @ vertices                  (message passing, per batch)
  Y   = mix_nl(cg(X, X))                 (per-node quadratic in X)
  S   = sum_j sph[:, j, :]               (neighbor sum commutes through the
  Z   = mix_rel(cg(Y, S))                 relative-CG stage: x-side is
  out = Z / sqrt(sum Z^2 / 16)            j-independent)

Device pipeline per core (bf16 engines, f32 PSUM accumulate):
  A:  X[i,144]     = matmul(lhsT=connT, rhs=vcat)
  B:  P[i,9984]    = 13 tensor_tensor product ops split DVE/GpSimd (dense
                     q-major slots, l1==l2 m-pairs folded by symmetry)
      PT[s,tp,i]   = 5 grouped XBAR dma_start_transposes of P (single HWDGE
                     queue: concurrent XBAR transposes corrupt; drain fences
                     give consumers completion ordering)
      YT[i, r]     = 78 matmuls lhsT=PT-chunk rhs=W2-chunk, PSUM-accumulated;
                     one PSUM bank per s-group so mixes run in transpose
                     completion order (interleaved open accumulation groups
                     sharing a bank break)
      Y rows       = XBAR transpose of YT back to (g,a)-partition layout
  S:  Ssum=reduce_j(sph) -> XBAR -> DRAM -> broadcast SREP128[.,n,i]
  C:  P2 = YTT * SREP (DVE 2x); Z[i,144] += P2_n.T @ W3_n  (18 matmuls)
Host epilogue: gather, unpack e=(l,c',k), global normalization per l.
"""
import numpy as np
from math import factorial, sqrt

MAXL = 2
CH = 16
NN = 128
NB = 8
LDIM = [1, 3, 5]
FOFF = [0, 16, 64]
NF = 144

# ------------------------------------------------------------- CG tables
def _cg_coeff(j1, m1, j2, m2, j3, m3):
    if m3 != m1 + m2:
        return 0.0
    pre = sqrt((2 * j3 + 1) * factorial(j3 + j1 - j2) * factorial(j3 - j1 + j2)
               * factorial(j1 + j2 - j3) / factorial(j1 + j2 + j3 + 1))
    pre *= sqrt(factorial(j3 + m3) * factorial(j3 - m3) * factorial(j1 - m1)
                * factorial(j1 + m1) * factorial(j2 - m2) * factorial(j2 + m2))
    s = 0.0
    vmin = max(0, j2 - j3 - m1, j1 - j3 + m2)
    vmax = min(j1 + j2 - j3, j1 - m1, j2 + m2)
    for v in range(vmin, vmax + 1):
        s += (-1) ** v / (factorial(v) * factorial(j1 + j2 - j3 - v)
                          * factorial(j1 - m1 - v) * factorial(j2 + m2 - v)
                          * factorial(j3 - j2 + m1 + v) * factorial(j3 - j1 - m2 + v))
    return pre * s


def _cg_matrix(l1, l2, l):
    M = np.zeros((2 * l1 + 1, 2 * l2 + 1, 2 * l + 1))
    for m1 in range(-l1, l1 + 1):
        for m2 in range(-l2, l2 + 1):
            if -l <= m1 + m2 <= l:
                M[m1 + l1, m2 + l2, m1 + m2 + l] = _cg_coeff(l1, m1, l2, m2, l, m1 + m2)
    return M


def _valid_pairs(l):
    return [(l1, l2) for l1 in range(3) for l2 in range(3)
            if abs(l1 - l2) <= l <= l1 + l2]

# ----------------------------------------------------- stage-B slot layout
def _build_q():
    qs = []
    for l1 in range(3):
        for l2 in range(l1, 3):
            for m1 in range(2 * l1 + 1):
                mt1 = m1 - l1
                m2_lo = max(0, -2 - mt1 + l2)
                m2_hi = min(2 * l2, 2 - mt1 + l2)
                if l1 == l2:
                    m2_lo = max(m2_lo, m1)
                if m2_lo > m2_hi:
                    continue
                qs.append((l1, l2, m1, m2_lo, m2_hi))
    return qs

QS = _build_q()                                          # 13 product ops
QOFF = np.concatenate([[0], np.cumsum([(q[4] - q[3] + 1) * 256 for q in QS])])
NSLOT = int(QOFF[-1])                                    # 9984
NCHUNK = NSLOT // 128                                    # 78

_DOFF = {}
_BLOCK_OF_CHUNK = []
CH_Q = []
for _k, (_l1, _l2, _m1, _lo, _hi) in enumerate(QS):
    for _m2 in range(_lo, _hi + 1):
        _DOFF[(_l1, _l2, _m1, _m2)] = int(QOFF[_k]) + (_m2 - _lo) * 256
        _BLOCK_OF_CHUNK += [(_l1, _l2, _m1, _m2)] * 2
        CH_Q += [_k, _k]
G_OF_CHUNK = [(m1 - l1) + (m2 - l2) + 2 for (l1, l2, m1, m2) in _BLOCK_OF_CHUNK]

# engine split and emission order (measured: DVE ~1.6 ns/col, Pool ~3.9)
DVE_QS = [2, 7, 6, 10, 3, 9, 1]
POOL_QS = [8, 11, 4, 0, 5, 12]
# XBAR transpose groups (engine, [q...]); concat per engine == emission order
TGROUPS = [(0, [2, 7]), (0, [6, 10]), (0, [3, 9, 1]), (1, [8, 11]), (1, [4, 0, 5, 12])]

def _nslots(k):
    return (QS[k][4] - QS[k][3] + 1) * 256

_PLOC = {}
_PSZ = [0, 0]
for _e, _qs in ((0, DVE_QS), (1, POOL_QS)):
    for _k in _qs:
        _PLOC[_k] = (_e, _PSZ[_e])
        _PSZ[_e] += _nslots(_k)

TBASE = {}
_tp = 0
for _e, _qs in TGROUPS:
    for _k in _qs:
        TBASE[_k] = _tp
        _tp += _nslots(_k) // 128
TPOS = [TBASE[CH_Q[c]] + (c - int(QOFF[CH_Q[c]]) // 128) for c in range(NCHUNK)]
MIX_ORDER = [c for _e, _qs in TGROUPS for _k in _qs
             for c in range(int(QOFF[_k]) // 128, int(QOFF[_k + 1]) // 128)]
_GPOS = {}
for _i, _c in enumerate(MIX_ORDER):
    _g = G_OF_CHUNK[_c]
    _GPOS.setdefault(_g, []).append(_i)
G_START = {g: v[0] for g, v in _GPOS.items()}
G_STOP = {g: v[-1] for g, v in _GPOS.items()}


def _sgroup_cols(g):
    st = g - 2
    return [(l, cp) for l in range(3) if abs(st) <= l for cp in range(CH)]

SG_NCOL = [len(_sgroup_cols(g)) for g in range(5)]      # [16,32,48,32,16]
YOFF = np.concatenate([[0], np.cumsum(SG_NCOL)])        # Y piece row offsets


def _sg_lblock_col(g, l):
    st = g - 2
    return 16 * sum(1 for lp in range(l) if abs(st) <= lp)

_CAR, _DAR = np.meshgrid(np.arange(16), np.arange(16), indexing="ij")


def _assemble_W2(w_nl):
    """W2[NSLOT, 48] f64: combined CG x w_nl, dense folded layout."""
    W2 = np.zeros((NSLOT, 48))
    for l in range(3):
        off = 0
        for (p1, p2) in _valid_pairs(l):
            Cg = _cg_matrix(p1, p2, l)
            wl = np.asarray(w_nl[l], np.float64)
            for m1 in range(2 * p1 + 1):
                for m2 in range(2 * p2 + 1):
                    st = (m1 - p1) + (m2 - p2)
                    if abs(st) > l:
                        continue
                    gc = Cg[m1, m2, st + l]
                    if gc == 0.0:
                        continue
                    g = st + 2
                    if p1 < p2 or (p1 == p2 and m1 <= m2):
                        slots = _DOFF[(p1, p2, m1, m2)] + _CAR * 16 + _DAR
                    else:
                        slots = _DOFF[(p2, p1, m2, m1)] + _DAR * 16 + _CAR
                    t = off + _CAR * 16 + _DAR
                    c0 = _sg_lblock_col(g, l)
                    W2[slots.ravel(), c0:c0 + 16] += gc * wl[t.ravel(), :]
            off += 256
    return W2


def _assemble_W3(w_rel):
    """W3[9*144, 144]: contraction P2[(n,r), i] -> Z[e, i]; r = Y piece row."""
    SOFF = [0, 1, 4]
    W3 = np.zeros((9 * 144, 144))
    ar = np.arange(16)
    for l in range(3):
        off = 0
        for (p1, p2) in _valid_pairs(l):
            Cg = _cg_matrix(p1, p2, l)
            wr = np.asarray(w_rel[l], np.float64)
            for m1 in range(2 * p1 + 1):
                for m2 in range(2 * p2 + 1):
                    st = (m1 - p1) + (m2 - p2)
                    if abs(st) > l:
                        continue
                    gc = Cg[m1, m2, st + l]
                    if gc == 0.0:
                        continue
                    gY = (m1 - p1) + 2
                    a0 = YOFF[gY] + _sg_lblock_col(gY, p1)
                    rows = (SOFF[p2] + m2) * 144 + a0 + ar
                    cols = FOFF[l] + (st + l) + ar * LDIM[l]
                    W3[np.ix_(rows, cols)] += gc * wr[off:off + 16, :]
            off += 16
    return W3

# ------------------------------------------------------------ bass builder
_NC_CACHE = {}


def _build_nc(debug=False):
    import concourse.bacc as bacc
    import concourse.bass as bass
    import concourse.tile as tile
    from concourse import mybir
    from concourse.tile import add_dep_helper

    def dep(a, b, why):
        # annotate_deps misses InstDmaTransposeAnt / hand-built AP operands;
        # wire edges explicitly (drain fences give DMA-completion ordering).
        add_dep_helper(a.ins, b.ins, reason=why)

    f32 = mybir.dt.float32
    bf16 = mybir.dt.bfloat16
    nc = bacc.Bacc()
    d_cv = nc.declare_dram_parameter("cvcat", [128, 128 + NF], bf16, isOutput=False)
    d_sph = nc.declare_dram_parameter("sph", [128, 128 * 9], bf16, isOutput=False)
    d_w2 = nc.declare_dram_parameter("w2", [128, NCHUNK * 48], bf16, isOutput=False)
    d_w3a = nc.declare_dram_parameter("w3a", [128, 9 * 144], bf16, isOutput=False)
    d_w3b = nc.declare_dram_parameter("w3b", [48, 9 * 144], bf16, isOutput=False)
    d_zout = nc.declare_dram_parameter("zout", [128, NF], f32, isOutput=True)
    if debug:
        d_dbgx = nc.declare_dram_parameter("dbgx", [128, NF], f32, isOutput=True)
        d_dbgs = nc.declare_dram_parameter("dbgs", [128, 9 * 128], f32, isOutput=True)
        d_dbgp = nc.declare_dram_parameter("dbgp", [128, NSLOT], f32, isOutput=True)
        d_dbgy = nc.declare_dram_parameter("dbgy", [128, 256], f32, isOutput=True)

    def vap(t, doff, freedims):
        base = t[:] if not isinstance(t, bass.AP) else t
        return bass.AP(tensor=base.tensor, offset=base.offset + doff,
                       ap=[list(base.ap[0])] + [list(d) for d in freedims])

    with tile.TileContext(nc) as tc:
      with (
        tc.tile_pool(name="big", bufs=1) as big,
        tc.tile_pool(name="sb", bufs=1) as sb,
        tc.tile_pool(name="ps_m", bufs=1, space="PSUM") as ps_m,
        tc.tile_pool(name="ps_g", bufs=1, space="PSUM") as ps_g,
      ):
        # ---- input DMAs on ACT queue (keeps the SP HWDGE free for XBAR)
        cv = sb.tile([128, 128 + NF], bf16)
        nc.scalar.dma_start(out=cv, in_=d_cv[:, :])

        # ---- stage A: X[i, feat] = connT.T @ vcat
        x_ps = ps_m.tile([128, NF], f32, tag="misc", name="x_ps")
        nc.tensor.matmul(x_ps, cv[:, 0:128], cv[:, 128:128 + NF],
                         start=True, stop=True)
        X = sb.tile([128, NF], bf16)
        nc.scalar.activation(X, x_ps, mybir.ActivationFunctionType.Copy)

        sph = big.tile([128, 128 * 9], bf16)   # [j, (n, i)] layout
        nc.scalar.dma_start(out=sph, in_=d_sph[:, :])
        w2 = big.tile([128, NCHUNK, 48], bf16)
        for q in range(2):
            lo = (NCHUNK * q // 2) * 48
            hi = (NCHUNK * (q + 1) // 2) * 48
            nc.scalar.dma_start(out=vap(w2, lo, [[1, hi - lo]]), in_=d_w2[:, lo:hi])
        w3a = sb.tile([128, 9, 144], bf16)
        nc.scalar.dma_start(
            out=w3a, in_=d_w3a[:, :].rearrange("p (n e) -> p n e", n=9))
        w3b = sb.tile([48, 9, 144], bf16)
        nc.scalar.dma_start(
            out=w3b, in_=d_w3b[:, :].rearrange("p (n e) -> p n e", n=9))

        # ---- stage B products: P split into per-engine tiles (DVE / GpSimd)
        Pt = [big.tile([128, _PSZ[0]], bf16, name="P_dve"),
              big.tile([128, _PSZ[1]], bf16, name="P_pool")]
        pinst = {}
        for e, qs in ((0, DVE_QS), (1, POOL_QS)):
            eng = nc.vector if e == 0 else nc.gpsimd
            for k in qs:
                l1, l2, m1, m2_lo, m2_hi = QS[k]
                nm2 = m2_hi - m2_lo + 1
                loc = _PLOC[k][1]
                pinst[k] = eng.tensor_tensor(
                    out=vap(Pt[e], loc, [[256, nm2], [16, 16], [1, 16]]),
                    in0=vap(X, FOFF[l1] + m1, [[0, nm2], [LDIM[l1], 16], [0, 16]]),
                    in1=vap(X, FOFF[l2] + m2_lo,
                            [[1, nm2], [0, 16], [LDIM[l2], 16]]),
                    op=mybir.AluOpType.mult)

        # ---- XBAR transposes (single SP queue). Same-queue DMA transfers
        # execute in order, so a tiny regular DMA ("sliver fence") right after
        # each transpose rewrites one element per chunk in place: tile tracks
        # it normally, giving the mixes completion-ordered RAW deps that
        # transitively cover the untracked XBAR write.
        PT = big.tile([128, NCHUNK, 128], bf16)
        gfence = []
        for e, qs in TGROUPS:
            lo = _PLOC[qs[0]][1]
            sz = sum(_nslots(k) for k in qs)
            t0 = TBASE[qs[0]]
            nch = sz // 128
            ti = nc.sync.dma_start_transpose(
                out=PT[:, t0:t0 + nch, :], in_=Pt[e][:, lo:lo + sz])
            for k in qs:
                dep(ti, pinst[k], "xbar reads P block")
            ap0 = list(PT[:].ap[0])
            ap0[1] = 1   # single partition: nch 2-byte descriptors only
            sliver = bass.AP(tensor=PT[:].tensor, offset=PT[:].offset + t0 * 128,
                             ap=[ap0, [128, nch]])
            fi = nc.sync.dma_start(out=sliver, in_=sliver)
            dep(fi, ti, "fence follows xbar on queue")
            gfence.append(fi)

        # ---- stage S on PE (idle window): S[n,i] = sum_j sph[j,(n,i)] via a
        # ones-matmul, then K=1 ones-matmul broadcasts it to 128 partitions.
        ones16 = sb.tile([128, 16], bf16)
        nc.gpsimd.memset(ones16, 1.0)
        ones1 = sb.tile([1, 128], bf16)
        nc.gpsimd.memset(ones1, 1.0)
        s_row = sb.tile([1, 9 * 128], bf16)
        SREP = sb.tile([128, 9, 128], bf16)
        for t in range(3):
            lo = t * 384
            s_ps = ps_m.tile([16, 384], f32, tag="sjs", name="s_ps")
            nc.tensor.matmul(s_ps, ones16, sph[:, lo:lo + 384],
                             start=True, stop=True)
            nc.scalar.activation(s_row[0:1, lo:lo + 384], s_ps[0:1, :],
                                 mybir.ActivationFunctionType.Copy)
            s_bc = ps_m.tile([128, 384], f32, tag="sbc", name="s_bc")
            nc.tensor.matmul(s_bc, ones1, s_row[0:1, lo:lo + 384],
                             start=True, stop=True)
            nc.scalar.activation(vap(SREP, lo, [[1, 384]]), s_bc,
                                 mybir.ActivationFunctionType.Copy)

        # ---- stage B mixes: Y rows stacked on PSUM partitions (bank
        # accumulation state is per-partition, so partition-disjoint groups
        # interleave freely). Matmul out base partition must be 0/32/64:
        # g0@A0, g1@A32, g2@A64, g3@B0, g4@B32; junk rows zeroed.
        ymA = ps_g.tile([128, 128], f32, name="ymA")
        ymB = ps_g.tile([48, 128], f32, name="ymB")
        # zero junk rows 16:32 and 112:128 (32-aligned ranges; runs before the
        # mixes' start=True overwrites, so touching real rows is harmless)
        nc.vector.memset(ymA[0:32, :], 0.0)
        nc.vector.memset(ymA[96:128, :], 0.0)
        G_ROW = {0: (0, 0), 1: (0, 32), 2: (0, 64), 3: (1, 0), 4: (1, 32)}
        for i, ch in enumerate(MIX_ORDER):
            g = G_OF_CHUNK[ch]
            ncol = SG_NCOL[g]
            t, base = G_ROW[g]
            out = (ymA if t == 0 else ymB)[base:base + ncol, :]
            mm = nc.tensor.matmul(
                out, w2[:, i, 0:ncol], PT[:, TPOS[ch], :],
                start=(i == G_START[g]), stop=(i == G_STOP[g]))
            if i in _MIXFENCE:
                dep(mm, gfence[_MIXFENCE[i]], "mix waits PT xbar drain")

        ysbA = sb.tile([128, 128], bf16)
        nc.scalar.activation(ysbA, ymA, mybir.ActivationFunctionType.Copy)
        ysbB = sb.tile([48, 128], bf16)
        nc.scalar.activation(ysbB, ymB, mybir.ActivationFunctionType.Copy)

        if debug:
            xdb = sb.tile([128, NF], f32)
            nc.vector.tensor_copy(out=xdb, in_=X)
            nc.sync.dma_start(out=d_dbgx[:, :], in_=xdb)
            sdb = big.tile([128, 9 * 128], f32)
            nc.vector.tensor_copy(out=sdb, in_=vap(SREP, 0, [[1, 9 * 128]]))
            nc.sync.dma_start(out=d_dbgs[:, :], in_=sdb)
            pdb = big.tile([128, NSLOT], f32)
            for k in range(len(QS)):
                e, loc = _PLOC[k]
                lo, hi = int(QOFF[k]), int(QOFF[k + 1])
                nc.vector.tensor_copy(out=vap(pdb, lo, [[1, hi - lo]]),
                                      in_=Pt[e][:, loc:loc + hi - lo])
            for q in range(4):
                lo = NSLOT * q // 4
                hi = NSLOT * (q + 1) // 4
                nc.sync.dma_start(out=d_dbgp[:, lo:hi],
                                  in_=vap(pdb, lo, [[1, hi - lo]]))
            ydb = sb.tile([128, 256], f32)
            nc.vector.tensor_copy(out=ydb[:, 0:128], in_=ysbA)
            nc.vector.tensor_copy(out=ydb[0:16, 128:256], in_=ysbB)
            nc.sync.dma_start(out=d_dbgy[:, :], in_=ydb)

        # ---- stage C: P2 = Y * SREP (DVE 2x); Z[i,144] += P2_n.T @ W3_n
        p2a = sb.tile([128, 9, 128], bf16)
        p2b = sb.tile([48, 9, 128], bf16)
        for n in range(9):
            nc.vector.tensor_tensor(
                out=p2a[:, n, :], in0=ysbA, in1=SREP[:, n, :],
                op=mybir.AluOpType.mult)
            nc.vector.tensor_tensor(
                out=p2b[:, n, :], in0=ysbB, in1=SREP[0:48, n, :],
                op=mybir.AluOpType.mult)
        zps = ps_m.tile([128, NF], f32, tag="misc", name="z_ps")
        for n in range(9):
            nc.tensor.matmul(zps, p2a[:, n, :], w3a[:, n, :],
                             start=(n == 0), stop=False)
            nc.tensor.matmul(zps, p2b[0:48, n, :], w3b[0:48, n, :],
                             start=False, stop=(n == 8))

        zs = sb.tile([128, NF], f32)
        nc.scalar.activation(zs, zps, mybir.ActivationFunctionType.Copy)
        nc.sync.dma_start(out=d_zout[:, :], in_=zs)

    nc.compile()
    return nc


# first mix position of each transpose group -> group fence index
_MIXFENCE = {}
_pos = 0
for _gi, (_e, _qs) in enumerate(TGROUPS):
    _MIXFENCE[_pos] = _gi
    _pos += sum(_nslots(_k) // 128 for _k in _qs)

# ------------------------------------------------------------- host entry
def _get_nc(debug=False):
    key = ("dbg" if debug else "nc")
    if key not in _NC_CACHE:
        _NC_CACHE[key] = _build_nc(debug)
    return _NC_CACHE[key]


def kernel(vertices_0, vertices_1, vertices_2, connectivity,
           sph_0, sph_1, sph_2,
           w_nl_0, w_nl_1, w_nl_2,
           w_rel_0, w_rel_1, w_rel_2, _debug=False):
    from concourse.bass_utils import run_bass_kernel_spmd
    import ml_dtypes

    f = np.float32
    bf = ml_dtypes.bfloat16
    verts = [np.asarray(v, f) for v in (vertices_0, vertices_1, vertices_2)]
    sphs = [np.asarray(s, f) for s in (sph_0, sph_1, sph_2)]
    conn = np.asarray(connectivity)
    W2 = _assemble_W2([np.asarray(w, f) for w in (w_nl_0, w_nl_1, w_nl_2)])
    W3 = _assemble_W3([np.asarray(w, f) for w in (w_rel_0, w_rel_1, w_rel_2)])
    # pack to SBUF-ready layouts (shared across cores); w2 chunks in MIX_ORDER
    w2p = np.ascontiguousarray(
        W2.reshape(NCHUNK, 128, 48)[MIX_ORDER].transpose(1, 0, 2)
        .reshape(128, NCHUNK * 48)).astype(bf)
    W3r = W3.reshape(9, 144, 144)
    # padded Y-row map: A rows [0:16]=g0, [32:64]=g1, [64:112]=g2 (junk rows
    # zeroed on device); B rows [0:32]=g3, [32:48]=g4
    w3a_r = np.zeros((128, 9, 144), np.float64)
    w3a_r[0:16] = W3r[:, 0:16, :].transpose(1, 0, 2)
    w3a_r[32:64] = W3r[:, 16:48, :].transpose(1, 0, 2)
    w3a_r[64:112] = W3r[:, 48:96, :].transpose(1, 0, 2)
    w3a = np.ascontiguousarray(w3a_r.reshape(128, 9 * 144)).astype(bf)
    w3b_r = W3r[:, 96:144, :].transpose(1, 0, 2)
    w3b = np.ascontiguousarray(w3b_r.reshape(48, 9 * 144)).astype(bf)

    in_maps = []
    for b in range(NB):
        connT = np.ascontiguousarray(conn[b].astype(f).T)
        vcat = np.concatenate([v[b].reshape(128, -1) for v in verts], axis=1)
        cvcat = np.concatenate([connT, vcat], axis=1).astype(bf)
        sph_cat = np.concatenate([s[b][:, :, 0, :] for s in sphs], axis=-1)
        sphT = sph_cat.transpose(1, 2, 0).reshape(128, 9 * 128)   # [j, (n, i)]
        in_maps.append(dict(cvcat=np.ascontiguousarray(cvcat),
                            sph=np.ascontiguousarray(sphT).astype(bf),
                            w2=w2p, w3a=w3a, w3b=w3b))

    res = run_bass_kernel_spmd(_get_nc(_debug), in_maps, list(range(NB)))
    if _debug:
        kernel._dbg = res
    Z = np.stack([res.results[b]["zout"] for b in range(NB)])   # [8, 128, 144]

    # host epilogue: unpack e=(l,cp,k) cols, global per-l normalization
    out = np.zeros((NB, 128, 1, 16, 9), dtype=f)
    koff = [0, 1, 4]
    for l in range(3):
        blk = Z[:, :, FOFF[l]:FOFF[l] + 16 * LDIM[l]]
        blk = blk.reshape(NB, 128, 16, LDIM[l])
        nf = np.sum(blk.astype(np.float64) ** 2)
        out[:, :, 0, :, koff[l]:koff[l] + LDIM[l]] = blk / np.sqrt(nf / 16.0)
    return out
